# revision 79
# baseline (speedup 1.0000x reference)
"""Multi-head causal self-attention (D=768, H=12, S=4096) on 8 Trainium2 cores.

Sharding: 4 head-groups (3 heads each) x 2 interleaved query-sets.
Core c = 2*g + s owns head-group g (heads 3g..3g+2) and query 128-row
blocks s, s+2, s+4, ... (even/odd interleave balances the causal
triangle).  Every core runs the SAME program; per-core behaviour is
driven entirely by input data (weight slices, gathered query columns,
multiplicative causal masks).  Each core produces a partial [2048, 768]
output (its heads pushed through its slice of Wo, all biases folded
in); the host sums the 4 group partials per query-set and re-interleaves
rows.

Per-core layout (v2):
  - scores computed transposed, S_T[k, q] = K_h Q_h^T, in [128k, 3h, 256q]
    PSUM tiles (2 banks, double buffered) -> ONE exp per tile.
  - PV is "flipped": stationary = P chunk [128k, 128q], moving = V block
    [128k, 65] (col 64 = ones -> softmax denominator), accumulating into
    per-query-chunk PSUM accumulators [128q, 3*65].  Cost 65 cycles per
    matmul instead of a full q-span.
  - each 128-query chunk finishes early (causal: chunk j stops at key
    block 8qg+2j+1): normalize with reciprocal + per-partition
    tensor_scalar, PE-transpose back to [hd, q], then out-projection and
    DMA of that 128-row output tile.
  - projections for group g+1 are interleaved into group g's attention
    m-loop; all inputs arrive partition-major so DMAs are few and big,
    ordered by first use.
"""

import numpy as np

D = 768
S = 4096
H = 12
HD = 64
NG = 4          # head groups
GH = 3          # heads per group
GD = GH * HD    # 192 dims per group
SL = S // 2     # local queries per core (2048)
P = 128
NC = D // P     # 6 contraction chunks
QG = 4          # query groups per core (512 q each)
QGS = 512
HS = 256        # half-span (scores tile q width)
NKB = S // P    # 32 key blocks
NQB = SL // P   # 16 local query tiles

_CACHE = {}

import os as _os
_BISECT = _os.environ.get("KERNEL_BISECT", "")


def _q0b(qg, m):
    """First active 128-query chunk (0..3) of group qg for key block m.
    Valid for both core parities (s=1 bound; leftovers via data mask)."""
    if m <= 0:
        return 0
    return max(0, -(-(m - 1) // 2) - 4 * qg)


def _build_program():
    from contextlib import nullcontext as _nullcm
    import concourse.bacc as bacc
    import concourse.mybir as mybir
    import concourse.tile as tile
    from contextlib import ExitStack

    bf16 = mybir.dt.bfloat16
    f32 = mybir.dt.float32

    nc = bacc.Bacc("TRN2", target_bir_lowering=False, debug=False, num_devices=8)

    # All inputs partition-major (p = 128 rows).
    xt = nc.dram_tensor("xt", [P, NC, S], bf16, kind="ExternalInput").ap()
    xtq = nc.dram_tensor("xtq", [P, NC, SL], bf16, kind="ExternalInput").ap()
    wq = nc.dram_tensor("wq", [P, NC, GD], bf16, kind="ExternalInput").ap()
    wk = nc.dram_tensor("wk", [P, NC, GD], bf16, kind="ExternalInput").ap()
    wv = nc.dram_tensor("wv", [P, NC, GD], bf16, kind="ExternalInput").ap()
    wota0 = nc.dram_tensor("wota0", [P, D], bf16, kind="ExternalInput").ap()
    wota1 = nc.dram_tensor("wota1", [65, D], bf16, kind="ExternalInput").ap()
    bias = nc.dram_tensor("bias", [P, 4], f32, kind="ExternalInput").ap()
    masks = nc.dram_tensor("masks", [P, 8 * QGS], bf16, kind="ExternalInput").ap()
    ident = nc.dram_tensor("ident", [P, P], bf16, kind="ExternalInput").ap()
    out = nc.dram_tensor("out", [SL, D], f32, kind="ExternalOutput").ap()

    Exp = mybir.ActivationFunctionType.Exp
    Copy = mybir.ActivationFunctionType.Copy
    mult = mybir.AluOpType.mult
    add = mybir.AluOpType.add

    with tile.TileContext(nc) as tc, ExitStack() as ctx:
        const = ctx.enter_context(tc.tile_pool(name="const", bufs=1))

        # ---- persistent SBUF tiles ----
        xt_sb = const.tile([P, NC, S], bf16, tag="xt")
        xtq_sb = const.tile([P, NC, SL], bf16, tag="xtq")
        wq_sb = const.tile([P, NC, GD], bf16, tag="wq")
        wk_sb = const.tile([P, NC, GD], bf16, tag="wk")
        wv_sb = const.tile([P, NC, GD], bf16, tag="wv")
        wota0_sb = const.tile([P, D], bf16, tag="wota0")
        wota1_sb = const.tile([65, D], bf16, tag="wota1")
        bias_sb = const.tile([P, 4], f32, tag="bias")
        mask_sb = const.tile([P, 8, QGS], bf16, tag="masks")
        ident_sb = const.tile([P, P], bf16, tag="ident")
        kt01_sb = const.tile([P, S], bf16, tag="kt01")   # heads 0,1 stacked
        kt2_sb = const.tile([64, S], bf16, tag="kt2")
        qt01_sb = const.tile([P, SL], bf16, tag="qt01")
        qt2_sb = const.tile([64, SL], bf16, tag="qt2")
        # V: [128 k-part, kb, 3h*65] with col 64 of each 65-group = 1.0
        v1_sb = const.tile([P, NKB, GH * 65], bf16, tag="v1")
        stk0_sb = const.tile([P, QGS], bf16, tag="stk0")  # att^T rows hd 0..127
        stk1_sb = const.tile([65, QGS], bf16, tag="stk1")  # hd 128..191 + ones row

        # ---- input DMAs, in first-use order ----
        mask_r = masks.rearrange("p (w x) -> p w x", x=QGS)
        nc.sync.dma_start(wq_sb[:], wq[:])
        nc.sync.dma_start(xtq_sb[:, 0:3, 0:HS], xtq[:, 0:3, 0:HS])
        nc.sync.dma_start(wk_sb[:], wk[:])
        nc.sync.dma_start(xtq_sb[:, 3:6, 0:HS], xtq[:, 3:6, 0:HS])
        nc.sync.dma_start(xt_sb[:, 0:3, 0:P], xt[:, 0:3, 0:P])
        nc.sync.dma_start(xt_sb[:, 3:6, 0:P], xt[:, 3:6, 0:P])
        nc.sync.dma_start(bias_sb[:], bias[:])
        nc.sync.dma_start(xtq_sb[:, :, HS:QGS], xtq[:, :, HS:QGS])
        nc.sync.dma_start(wv_sb[:], wv[:])
        nc.sync.dma_start(mask_sb[:, 0:1, :], mask_r[:, 0:1, :])
        nc.sync.dma_start(ident_sb[:], ident[:])
        nc.sync.dma_start(xt_sb[:, :, P:QGS], xt[:, :, P:QGS])
        nc.sync.dma_start(mask_sb[:, 1:4, :], mask_r[:, 1:4, :])
        nc.sync.dma_start(xt_sb[:, :, QGS:2 * QGS], xt[:, :, QGS:2 * QGS])
        nc.sync.dma_start(xtq_sb[:, :, QGS:2 * QGS], xtq[:, :, QGS:2 * QGS])
        nc.sync.dma_start(mask_sb[:, 4:8, :], mask_r[:, 4:8, :])
        nc.sync.dma_start(wota0_sb[:], wota0[:])
        nc.sync.dma_start(wota1_sb[:], wota1[:])
        for g in range(1, QG):
            if g > 1:
                nc.sync.dma_start(
                    xtq_sb[:, :, g * QGS:(g + 1) * QGS],
                    xtq[:, :, g * QGS:(g + 1) * QGS])
            nc.sync.dma_start(
                xt_sb[:, :, 2 * g * QGS:(2 * g + 1) * QGS],
                xt[:, :, 2 * g * QGS:(2 * g + 1) * QGS])
            nc.sync.dma_start(
                xt_sb[:, :, (2 * g + 1) * QGS:(2 * g + 2) * QGS],
                xt[:, :, (2 * g + 1) * QGS:(2 * g + 2) * QGS])

        # ones columns of V (denominator) and of stk1 (bias row of out-proj)
        v1_v = v1_sb[:].rearrange("p k (h x) -> p k h x", x=65)
        nc.vector.memset(v1_v[:, :, :, 64:65], 1.0)
        nc.vector.memset(stk1_sb[64:65, :], 1.0)
        warm_sb = const.tile([1, QGS], bf16, tag="warm")
        nc.vector.memset(warm_sb[:], 0.0)

        # Slot order [head0, head2, head1]: the two base-partition-0 heads
        # share the first PSUM bank of the scores tile; the base-64 head
        # (kt01/qt01 rows 64:128) gets the second bank alone.  Matmuls that
        # share a PSUM bank must use the same operand base partition (HW
        # exec-unit constraint); V and Wo are permuted on the host to match.
        kq = [(kt01_sb, 0), (kt2_sb, 0), (kt01_sb, 64)]
        qq = [(qt01_sb, 0), (qt2_sb, 0), (qt01_sb, 64)]

        with tc.tile_pool(name="st_ps", bufs=2, space="PSUM") as stps, \
             tc.tile_pool(name="pv_ps", bufs=1, space="PSUM") as pvps, \
             tc.tile_pool(name="aux_ps", bufs=2, space="PSUM") as auxps, \
             tc.tile_pool(name="pt", bufs=8) as ptpool, \
             tc.tile_pool(name="nrm", bufs=4) as nrmpool, \
             tc.tile_pool(name="attn", bufs=4) as attnpool, \
             tc.tile_pool(name="oev", bufs=4) as oevpool:

            # per-group PV accumulators (persist across the m-loop)
            pvA = pvps.tile([P, QGS], f32, tag="pvA", name="pvA")  # chunks 0,1
            pvB = pvps.tile([P, QGS], f32, tag="pvB", name="pvB")  # chunks 2,3

            # ---------- projection pieces (emitted interleaved) ----------
            def qk_chain(w_sb, x_sb, x0, dst, bias_col, rows, wlo, name,
                         width=QGS, hoist=0):
                """Returns 2 sub-lump closures (3+3 matmuls) for one
                projection chain, so pacing never inserts a >700ns PE lump."""
                state = {}

                def p1():
                    with tc.high_priority(hoist) if hoist else _nullcm():
                        ps = auxps.tile([P, width], f32, tag="aux", name=name)
                        state["ps"] = ps
                        for c in range(3):
                            nc.tensor.matmul(
                                ps[0:rows, :], w_sb[:, c, wlo:wlo + rows],
                                x_sb[:, c, x0:x0 + width],
                                start=(c == 0), stop=False)

                def p2():
                    with tc.high_priority(hoist) if hoist else _nullcm():
                        ps = state["ps"]
                        for c in range(3, NC):
                            nc.tensor.matmul(
                                ps[0:rows, :], w_sb[:, c, wlo:wlo + rows],
                                x_sb[:, c, x0:x0 + width],
                                start=False, stop=(c == NC - 1))
                    # the evict gates downstream scores: keep it ahead of
                    # other DVE work
                    with tc.high_priority(150):
                        nc.vector.tensor_scalar(dst, ps[0:rows, :], bias_col,
                                                None, add)
                return [p1, p2]

            def q_subs(g, part):
                if part == 0:
                    return qk_chain(wq_sb, xtq_sb, g * QGS,
                                    qt01_sb[:, g * QGS:(g + 1) * QGS],
                                    bias_sb[:, 0:1], 128, 0, f"q{g}_0")
                return qk_chain(wq_sb, xtq_sb, g * QGS,
                                qt2_sb[:, g * QGS:(g + 1) * QGS],
                                bias_sb[0:64, 1:2], 64, 128, f"q{g}_1")

            def k_subs(kg, part):
                if part == 0:
                    return qk_chain(wk_sb, xt_sb, kg * QGS,
                                    kt01_sb[:, kg * QGS:(kg + 1) * QGS],
                                    bias_sb[:, 2:3], 128, 0, f"k{kg}_0")
                return qk_chain(wk_sb, xt_sb, kg * QGS,
                                kt2_sb[:, kg * QGS:(kg + 1) * QGS],
                                bias_sb[0:64, 3:4], 64, 128, f"k{kg}_1")

            def v_piece(kb):
                ps = auxps.tile([P, GD], f32, tag="aux", name=f"v{kb}")
                for c in range(NC):
                    nc.tensor.matmul(
                        ps[:], xt_sb[:, c, kb * P:(kb + 1) * P], wv_sb[:, c, :],
                        start=(c == 0), stop=(c == NC - 1))
                dst = v1_v[:, kb, :, 0:64]
                src = ps[:].rearrange("p (h x) -> p h x", x=64)
                nc.vector.tensor_copy(dst, src)

            # ---------- per-chunk finish: normalize, transpose, out-proj ----------
            def finish_head(qg, j):
                """Reads the chunk's PSUM accumulator: normalize, transpose,
                stack.  MUST be emitted before the next group re-opens the
                bank (WAR)."""
                half, jj = j // 2, j % 2
                pvt = pvA if half == 0 else pvB
                joff = 256 * jj
                view = pvt[:, joff:joff + GH * 65].rearrange(
                    "p (h x) -> p h x", x=65)
                rc = nrmpool.tile([P, GH, 1], f32, tag="rc")
                nc.vector.reciprocal(rc[:], view[:, :, 64:65])
                att = attnpool.tile([P, GD], bf16, tag="attn")
                for h in range(GH):
                    nc.vector.tensor_scalar(
                        att[:, h * HD:(h + 1) * HD], view[:, h, 0:64],
                        rc[:, h, :], None, mult)
                tp = auxps.tile([P, 2 * P], bf16, tag="aux", name=f"tp{qg}_{j}")
                nc.tensor.transpose(tp[:, 0:P], att[:, 0:P], ident_sb[:])
                nc.tensor.transpose(tp[0:64, P:2 * P], att[:, P:GD], ident_sb[:])
                # last group: Act is idle, shorten the tail-critical chain
                if qg == QG - 1:
                    nc.scalar.activation(stk0_sb[:, j * P:(j + 1) * P],
                                         tp[:, 0:P], Copy)
                else:
                    nc.vector.tensor_copy(stk0_sb[:, j * P:(j + 1) * P],
                                          tp[:, 0:P])
                nc.vector.tensor_copy(stk1_sb[0:64, j * P:(j + 1) * P],
                                      tp[0:64, P:2 * P])

            def finish_tail(qg, j, dh):
                """Out-projection half dh for query tile j (reads stk only)."""
                oe_key = (qg, j)
                if oe_key not in oe_tiles:
                    oe_tiles[oe_key] = oevpool.tile([P, D], f32, tag="oe",
                                                    name=f"oe{qg}_{j}")
                oe = oe_tiles[oe_key]
                jq = 4 * qg + j
                op = auxps.tile([P, QGS], f32, tag="aux", name=f"op{qg}_{j}_{dh}")
                nc.tensor.matmul(
                    op[:, 0:384], stk0_sb[:, j * P:(j + 1) * P],
                    wota0_sb[:, dh * 384:(dh + 1) * 384],
                    start=True, stop=False)
                nc.tensor.matmul(
                    op[:, 0:384], stk1_sb[:, j * P:(j + 1) * P],
                    wota1_sb[:, dh * 384:(dh + 1) * 384],
                    start=False, stop=True)
                if qg == QG - 1 and dh == 1:
                    nc.scalar.activation(oe[:, dh * 384:(dh + 1) * 384],
                                         op[:, 0:384], Copy)
                else:
                    nc.vector.tensor_copy(oe[:, dh * 384:(dh + 1) * 384],
                                          op[:, 0:384])
                nc.sync.dma_start(
                    out[jq * P:(jq + 1) * P, dh * 384:(dh + 1) * 384],
                    oe[:, dh * 384:(dh + 1) * 384])

            oe_tiles = {}

            # ---------- main loop ----------
            # PE warm-up: the p-state ramp needs ~3us of continuous work to
            # reach full clock; burn it on dummy matmuls while DMAs stream so
            # the first real projection chains run at full speed.
            for i in range(3):
                wps = auxps.tile([1, QGS], f32, tag="aux", name=f"warm{i}")
                nc.tensor.matmul(wps[:], warm_sb[0:1, 0:1], warm_sb[:],
                                 start=True, stop=True)

            # Upfront projections: only the slices the FIRST scores tile needs
            # (Q cols 0:256, K cols 0:128), so the first exp issues ~6us in.
            # The rest streams in via unit pre/post slots below.
            for fn in (qk_chain(wq_sb, xtq_sb, 0, qt01_sb[:, 0:HS],
                                bias_sb[:, 0:1], 128, 0, "qa0", HS)
                       + qk_chain(wq_sb, xtq_sb, 0, qt2_sb[:, 0:HS],
                                  bias_sb[0:64, 1:2], 64, 128, "qa1", HS)
                       + qk_chain(wk_sb, xt_sb, 0, kt01_sb[:, 0:P],
                                  bias_sb[:, 2:3], 128, 0, "ka0", P)
                       + qk_chain(wk_sb, xt_sb, 0, kt2_sb[:, 0:P],
                                  bias_sb[0:64, 3:4], 64, 128, "ka1", P)):
                fn()

            # Build the full tile sequence with per-tile emission closures, then
            # emit with a one-stage software-pipeline skew: scores+exp of tile
            # i+1 go into the engine queues BEFORE mask+PV of tile i, so the
            # in-order PE queue never parks on a PV that waits for its exp.
            units = []   # list of dicts: front(), back(), post list

            def make_front(qg, m, half, lo, holder):
                def front():
                    # scores+exp are the pacing stream: keep them ahead of
                    # filler in the scheduler's priority heap so a ready
                    # scores matmul is never queued behind projection lumps
                    with tc.high_priority(800):
                        qbase = qg * QGS + half * HS
                        st = stps.tile([P, GH, HS], f32, tag="st",
                                       name=f"st{qg}_{m}_{half}")
                        for h in range(GH):
                            kt_t, kp = kq[h]
                            qt_t, qp = qq[h]
                            nc.tensor.matmul(
                                st[:, h, lo:], kt_t[kp:kp + 64, m * P:(m + 1) * P],
                                qt_t[qp:qp + 64, qbase + lo:qbase + HS],
                                start=True, stop=True)
                        pt = ptpool.tile([P, GH, HS], bf16, tag="pt")
                        nc.scalar.activation(pt[:, :, lo:], st[:, :, lo:], Exp)
                    holder.append(pt)
                return front

            def make_back(qg, m, half, jlo, jhi, q0b, holder):
                def back():
                    pt = holder[0]
                    w = m - 8 * qg
                    if w >= 0:
                        # multiplicative 0/1 causal mask on boundary blocks;
                        # hoisted above other DVE work (it gates this tile's PV)
                        glo = max(P * q0b, half * HS)
                        ghi = min(P * (w // 2 + 1), half * HS + HS)
                        if glo < ghi:
                            a, b = glo - half * HS, ghi - half * HS
                            with tc.high_priority(150):
                                for h in range(GH):
                                    nc.vector.tensor_tensor(
                                        pt[:, h, a:b], pt[:, h, a:b],
                                        mask_sb[:, w, glo:ghi], mult)
                    pvt = pvA if half == 0 else pvB
                    for j in range(jlo, jhi + 1):
                        jj = j - 2 * half
                        for h in range(GH):
                            # one accumulation group per PSUM bank: start only
                            # on the bank's very first write (it marks the
                            # whole 2KB zero-region), stop only on the bank's
                            # very last write.
                            first = (m == 0 and j == 2 * half and h == 0)
                            last = (j == 2 * half + 1 and h == GH - 1
                                    and m == 8 * qg + 2 * j + 1)
                            nc.tensor.matmul(
                                pvt[:, 256 * jj + 65 * h:256 * jj + 65 * h + 65],
                                pt[:, h, jj * P:(jj + 1) * P],
                                v1_sb[:, m, 65 * h:65 * h + 65],
                                start=first, stop=last)
                return back

            # pass 1: enumerate tiles
            idx_of = {}
            for qg in range(QG):
                kcnt = 8 * (qg + 1)
                for m in range(kcnt):
                    q0b = _q0b(qg, m)
                    for half in range(2):
                        jlo = max(2 * half, q0b)
                        jhi = 2 * half + 1
                        if jlo > jhi:
                            continue
                        lo = P * (jlo - 2 * half)
                        holder = []
                        units.append({
                            "front": make_front(qg, m, half, lo, holder),
                            "back": make_back(qg, m, half, jlo, jhi, q0b, holder),
                            "pre": [],
                            "post": [],
                        })
                    idx_of[(qg, m)] = len(units) - 1

            # pass 2: attach finishes and spread projection pieces
            def attach(idx, fn):
                units[min(idx, len(units) - 1)]["post"].append(fn)

            for qg in range(QG):
                # pvA closes mid-group; spread its finish work over the next
                # units so no single PE lump starves the Act exp stream.
                closeA = 8 * qg + 3 + (2 if qg == 0 else 0)
                ia = idx_of[(qg, closeA)]
                attach(ia, lambda qg=qg: finish_head(qg, 0))
                attach(ia + 1, lambda qg=qg: finish_head(qg, 1))
                for k, (j, dh) in enumerate([(0, 0), (0, 1), (1, 0), (1, 1)]):
                    attach(ia + 2 + k,
                           lambda qg=qg, j=j, dh=dh: finish_tail(qg, j, dh))
                # pvB closes at group end.  Its heads must still be emitted
                # before the NEXT group's first half-1 PV re-opens the bank,
                # i.e. no later than the post of unit (g+1, m=0, half0).
                ib = idx_of[(qg, 8 * qg + 7)]
                attach(ib + 1, lambda qg=qg: finish_head(qg, 2))
                attach(ib + 1, lambda qg=qg: finish_head(qg, 3))
                for k, (j, dh) in enumerate([(2, 0), (2, 1), (3, 0), (3, 1)]):
                    attach(ib + 2 + k,
                           lambda qg=qg, j=j, dh=dh: finish_tail(qg, j, dh))

            # group-boundary blackout: no filler right where the next group's
            # first scores must reach the PE queue head
            black = set()
            for g in range(QG):
                ib = idx_of[(g, 8 * g + 7)]
                black |= {ib - 1, ib, ib + 1}

            def spread(pieces, i0, i1):
                """Distribute sub-lumps uniformly over unit posts [i0, i1],
                respecting per-piece deadlines and boundary blackouts."""
                n = len(pieces)
                for k, (fn, dl) in enumerate(pieces):
                    u = min(i0 + (k * (i1 - i0 + 1)) // n, dl)
                    while u in black and u > i0:
                        u -= 1
                    units[u]["post"].append(fn)

            # startup remainder chains in unit pre-slots (pre of unit i runs
            # right after front(i), i.e. before front(i+1) and back(i)):
            # Q cols 256:512 after tile 0's scores; K cols 128:256 before m=1;
            # K cols 256:512 before m=2; V(kb) before back of m=kb.
            units[0]["pre"] += (
                qk_chain(wq_sb, xtq_sb, HS, qt01_sb[:, HS:QGS],
                         bias_sb[:, 0:1], 128, 0, "qb0", HS)
                + qk_chain(wq_sb, xtq_sb, HS, qt2_sb[:, HS:QGS],
                           bias_sb[0:64, 1:2], 64, 128, "qb1", HS))
            units[1]["pre"] += (
                qk_chain(wk_sb, xt_sb, P, kt01_sb[:, P:2 * P],
                         bias_sb[:, 2:3], 128, 0, "kb0", P)
                + qk_chain(wk_sb, xt_sb, P, kt2_sb[:, P:2 * P],
                           bias_sb[0:64, 3:4], 64, 128, "kb1", P)
                + [lambda: v_piece(0)])
            units[3]["pre"] += (
                qk_chain(wk_sb, xt_sb, 2 * P, kt01_sb[:, 2 * P:QGS],
                         bias_sb[:, 2:3], 128, 0, "kc0", 2 * P)
                + qk_chain(wk_sb, xt_sb, 2 * P, kt2_sb[:, 2 * P:QGS],
                           bias_sb[0:64, 3:4], 64, 128, "kc1", 2 * P)
                + [lambda: v_piece(1)])
            units[5]["pre"].append(lambda: v_piece(2))
            units[7]["pre"].append(lambda: v_piece(3))

            # group 0's remaining K/V: held until their xt/wv DMAs land
            own0 = [(s, idx_of[(0, 3)]) for s in k_subs(1, 0) + k_subs(1, 1)]
            own0 += [(lambda kb=kb: v_piece(kb), idx_of[(0, kb - 1)])
                     for kb in range(4, 8)]
            spread(own0, idx_of[(0, 1)] + 1, idx_of[(0, 5)])

            # groups 1..3: K/V spread from late in the previous group across
            # this group's pre-boundary iterations (window start respects the
            # xt DMA arrival order); per-piece deadline = first consumer.
            kv_start = {1: (1, 4), 2: (2, 0), 3: (3, 0)}
            for g in range(1, QG):
                pieces = [(s, idx_of[(g, 8 * g - 1)])
                          for s in k_subs(2 * g, 0) + k_subs(2 * g, 1)]
                pieces += [(s, idx_of[(g, 8 * g + 3)])
                           for s in k_subs(2 * g + 1, 0) + k_subs(2 * g + 1, 1)]
                pieces += [(lambda kb=kb: v_piece(kb), idx_of[(g, kb - 1)])
                           for kb in range(8 * g, 8 * g + 8)]
                sg, sm = kv_start[g]
                spread(pieces, idx_of[(sg, sm)], idx_of[(g, 8 * g + 2)])
                # Q(g) as soon as its xtq DMA has landed — far away from the
                # group boundary so the first scores of group g issue promptly
                q0g, q0m, q1m = {1: (0, 5, 6), 2: (1, 2, 5), 3: (2, 0, 3)}[g]
                qp = [(s, idx_of[(q0g, q1m + 1)])
                      for s in q_subs(g, 0) + q_subs(g, 1)]
                spread(qp, idx_of[(q0g, q0m)], idx_of[(q0g, q1m)])

            # skewed emission
            prev = None
            for u in units:
                u["front"]()
                for fn in u["pre"]:
                    fn()
                if prev is not None:
                    prev["back"]()
                    for fn in prev["post"]:
                        fn()
                prev = u
            prev["back"]()
            for fn in prev["post"]:
                fn()

    nc.compile()
    return nc


def _host_prep(inputs, Wq, bq, Wk, bk, Wv, bv, Wo, bo):
    import ml_dtypes

    bf16 = ml_dtypes.bfloat16
    X = np.asarray(inputs, np.float32).reshape(S, D)
    XT = np.ascontiguousarray(X.T)                      # [768, 4096]
    # partition-major: [128, 6, S]
    XTp = np.ascontiguousarray(
        XT.reshape(NC, P, S).transpose(1, 0, 2)).astype(bf16)
    XTb = XT.reshape(D, NKB // 2, 2, P)
    XTqp = []
    for s_ in range(2):
        XTq = XTb[:, :, s_, :].reshape(D, SL)
        XTqp.append(np.ascontiguousarray(
            XTq.reshape(NC, P, SL).transpose(1, 0, 2)).astype(bf16))

    # per-core multiplicative causal masks [128, 8*512], 1=keep 0=drop
    # (S_T layout: k on partitions, q on free dim)
    tri = (np.arange(P)[None, :] >= np.arange(P)[:, None]).astype(np.float32)
    mk = []
    for s_ in range(2):
        m = np.ones((P, 8, QGS), np.float32)
        for w in range(8):
            npref = max(0, -(-(w - s_) // 2))  # ceil((w - s)/2) clamped at 0
            m[:, w, :P * npref] = 0.0
            if w >= s_ and (w - s_) % 2 == 0:
                dblk = (w - s_) // 2
                m[:, w, dblk * P:(dblk + 1) * P] = tri
        mk.append(np.ascontiguousarray(m.reshape(P, 8 * QGS)).astype(bf16))

    idm = np.eye(P, dtype=np.float32).astype(bf16)

    def pmajor(WT):  # [768, 192] -> [128, 6, 192]
        return np.ascontiguousarray(
            WT.reshape(NC, P, GD).transpose(1, 0, 2)).astype(bf16)

    in_maps = []
    for g in range(NG):
        hs = slice(GD * g, GD * (g + 1))
        WqT = pmajor(np.ascontiguousarray(Wq[hs, :].T) / 8.0)
        WkT = pmajor(np.ascontiguousarray(Wk[hs, :].T))
        # slot order [head0, head2, head1] (see kq/qq comment in the program)
        sperm = np.r_[0:64, 128:192, 64:128]
        WvT = pmajor(np.ascontiguousarray(Wv[hs, :].T[:, sperm]))
        WoT = np.ascontiguousarray(Wo[:, hs].T[sperm, :]).astype(np.float32)
        bo_g = bv[hs][sperm].astype(np.float32) @ WoT
        if g == 0:
            bo_g = bo_g + bo.astype(np.float32)
        wota = np.concatenate([WoT, bo_g[None, :]], axis=0)  # [193, 768]
        wota0 = np.ascontiguousarray(wota[0:P]).astype(bf16)
        wota1 = np.ascontiguousarray(wota[P:]).astype(bf16)
        bias_t = np.zeros((P, 4), np.float32)
        bias_t[:, 0] = bq[hs][0:128] / 8.0
        bias_t[0:64, 1] = bq[hs][128:192] / 8.0
        bias_t[:, 2] = bk[hs][0:128]
        bias_t[0:64, 3] = bk[hs][128:192]
        for s_ in range(2):
            in_maps.append({
                "xt": XTp, "xtq": XTqp[s_],
                "wq": WqT, "wk": WkT, "wv": WvT,
                "wota0": wota0, "wota1": wota1,
                "bias": bias_t, "masks": mk[s_], "ident": idm,
            })
    return in_maps


def _gather(results):
    out = np.zeros((S, D), np.float32)
    ov = out.reshape(NQB, 2, P, D)
    for s_ in range(2):
        acc = np.zeros((SL, D), np.float32)
        for g in range(NG):
            acc += np.asarray(results[2 * g + s_]["out"], np.float32)
        ov[:, s_, :, :] = acc.reshape(NQB, P, D)
    return out.reshape(1, S, D)


def kernel(inputs, Wq, bq, Wk, bk, Wv, bv, Wo, bo):
    from concourse.bass_utils import run_bass_kernel_spmd

    if "nc" not in _CACHE:
        _CACHE["nc"] = _build_program()
    nc = _CACHE["nc"]
    in_maps = _host_prep(
        np.asarray(inputs), np.asarray(Wq), np.asarray(bq), np.asarray(Wk),
        np.asarray(bk), np.asarray(Wv), np.asarray(bv), np.asarray(Wo),
        np.asarray(bo))
    res = run_bass_kernel_spmd(nc, in_maps, list(range(8))).results
    return _gather(res)


# revision 83
# speedup vs baseline: 1.0049x; 1.0049x over previous
"""Multi-head causal self-attention (D=768, H=12, S=4096) on 8 Trainium2 cores.

Sharding: 4 head-groups (3 heads each) x 2 interleaved query-sets.
Core c = 2*g + s owns head-group g (heads 3g..3g+2) and query 128-row
blocks s, s+2, s+4, ... (even/odd interleave balances the causal
triangle).  Every core runs the SAME program; per-core behaviour is
driven entirely by input data (weight slices, gathered query columns,
multiplicative causal masks).  Each core produces a partial [2048, 768]
output (its heads pushed through its slice of Wo, all biases folded
in); the host sums the 4 group partials per query-set and re-interleaves
rows.

Per-core layout (v2):
  - scores computed transposed, S_T[k, q] = K_h Q_h^T, in [128k, 3h, 256q]
    PSUM tiles (2 banks, double buffered) -> ONE exp per tile.
  - PV is "flipped": stationary = P chunk [128k, 128q], moving = V block
    [128k, 65] (col 64 = ones -> softmax denominator), accumulating into
    per-query-chunk PSUM accumulators [128q, 3*65].  Cost 65 cycles per
    matmul instead of a full q-span.
  - each 128-query chunk finishes early (causal: chunk j stops at key
    block 8qg+2j+1): normalize with reciprocal + per-partition
    tensor_scalar, PE-transpose back to [hd, q], then out-projection and
    DMA of that 128-row output tile.
  - projections for group g+1 are interleaved into group g's attention
    m-loop; all inputs arrive partition-major so DMAs are few and big,
    ordered by first use.
"""

import numpy as np

D = 768
S = 4096
H = 12
HD = 64
NG = 4          # head groups
GH = 3          # heads per group
GD = GH * HD    # 192 dims per group
SL = S // 2     # local queries per core (2048)
P = 128
NC = D // P     # 6 contraction chunks
QG = 4          # query groups per core (512 q each)
QGS = 512
HS = 256        # half-span (scores tile q width)
NKB = S // P    # 32 key blocks
NQB = SL // P   # 16 local query tiles

_CACHE = {}

import os as _os
_BISECT = _os.environ.get("KERNEL_BISECT", "")


def _q0b(qg, m):
    """First active 128-query chunk (0..3) of group qg for key block m.
    Valid for both core parities (s=1 bound; leftovers via data mask)."""
    if m <= 0:
        return 0
    return max(0, -(-(m - 1) // 2) - 4 * qg)


def _build_program():
    from contextlib import nullcontext as _nullcm
    import concourse.bacc as bacc
    import concourse.mybir as mybir
    import concourse.tile as tile
    from contextlib import ExitStack

    bf16 = mybir.dt.bfloat16
    f32 = mybir.dt.float32

    nc = bacc.Bacc("TRN2", target_bir_lowering=False, debug=False, num_devices=8)

    # All inputs partition-major (p = 128 rows).
    xt = nc.dram_tensor("xt", [P, NC, S], bf16, kind="ExternalInput").ap()
    xtq = nc.dram_tensor("xtq", [P, NC, SL], bf16, kind="ExternalInput").ap()
    wq = nc.dram_tensor("wq", [P, NC, GD], bf16, kind="ExternalInput").ap()
    wk = nc.dram_tensor("wk", [P, NC, GD], bf16, kind="ExternalInput").ap()
    wv = nc.dram_tensor("wv", [P, NC, GD], bf16, kind="ExternalInput").ap()
    wota0 = nc.dram_tensor("wota0", [P, D], bf16, kind="ExternalInput").ap()
    wota1 = nc.dram_tensor("wota1", [65, D], bf16, kind="ExternalInput").ap()
    bias = nc.dram_tensor("bias", [P, 4], f32, kind="ExternalInput").ap()
    masks = nc.dram_tensor("masks", [P, 8 * QGS], bf16, kind="ExternalInput").ap()
    ident = nc.dram_tensor("ident", [P, P], bf16, kind="ExternalInput").ap()
    out = nc.dram_tensor("out", [SL, D], f32, kind="ExternalOutput").ap()

    Exp = mybir.ActivationFunctionType.Exp
    Copy = mybir.ActivationFunctionType.Copy
    mult = mybir.AluOpType.mult
    add = mybir.AluOpType.add

    with tile.TileContext(nc) as tc, ExitStack() as ctx:
        const = ctx.enter_context(tc.tile_pool(name="const", bufs=1))

        # ---- persistent SBUF tiles ----
        xt_sb = const.tile([P, NC, S], bf16, tag="xt")
        xtq_sb = const.tile([P, NC, SL], bf16, tag="xtq")
        wq_sb = const.tile([P, NC, GD], bf16, tag="wq")
        wk_sb = const.tile([P, NC, GD], bf16, tag="wk")
        wv_sb = const.tile([P, NC, GD], bf16, tag="wv")
        wota0_sb = const.tile([P, D], bf16, tag="wota0")
        wota1_sb = const.tile([65, D], bf16, tag="wota1")
        bias_sb = const.tile([P, 4], f32, tag="bias")
        mask_sb = const.tile([P, 8, QGS], bf16, tag="masks")
        ident_sb = const.tile([P, P], bf16, tag="ident")
        kt01_sb = const.tile([P, S], bf16, tag="kt01")   # heads 0,1 stacked
        kt2_sb = const.tile([64, S], bf16, tag="kt2")
        qt01_sb = const.tile([P, SL], bf16, tag="qt01")
        qt2_sb = const.tile([64, SL], bf16, tag="qt2")
        # V: [128 k-part, kb, 3h*65] with col 64 of each 65-group = 1.0
        v1_sb = const.tile([P, NKB, GH * 65], bf16, tag="v1")
        stk0_sb = const.tile([P, QGS], bf16, tag="stk0")  # att^T rows hd 0..127
        stk1_sb = const.tile([65, QGS], bf16, tag="stk1")  # hd 128..191 + ones row

        # ---- input DMAs, in first-use order ----
        mask_r = masks.rearrange("p (w x) -> p w x", x=QGS)
        nc.sync.dma_start(wq_sb[:], wq[:])
        nc.sync.dma_start(xtq_sb[:, 0:3, 0:HS], xtq[:, 0:3, 0:HS])
        nc.sync.dma_start(wk_sb[:], wk[:])
        nc.sync.dma_start(xtq_sb[:, 3:6, 0:HS], xtq[:, 3:6, 0:HS])
        nc.sync.dma_start(xt_sb[:, 0:3, 0:P], xt[:, 0:3, 0:P])
        nc.sync.dma_start(xt_sb[:, 3:6, 0:P], xt[:, 3:6, 0:P])
        nc.sync.dma_start(bias_sb[:], bias[:])
        nc.sync.dma_start(xtq_sb[:, :, HS:QGS], xtq[:, :, HS:QGS])
        nc.sync.dma_start(wv_sb[:], wv[:])
        nc.sync.dma_start(mask_sb[:, 0:1, :], mask_r[:, 0:1, :])
        nc.sync.dma_start(ident_sb[:], ident[:])
        nc.sync.dma_start(xt_sb[:, :, P:QGS], xt[:, :, P:QGS])
        nc.sync.dma_start(mask_sb[:, 1:4, :], mask_r[:, 1:4, :])
        nc.sync.dma_start(xt_sb[:, :, QGS:2 * QGS], xt[:, :, QGS:2 * QGS])
        nc.sync.dma_start(xtq_sb[:, :, QGS:2 * QGS], xtq[:, :, QGS:2 * QGS])
        nc.sync.dma_start(mask_sb[:, 4:8, :], mask_r[:, 4:8, :])
        nc.sync.dma_start(wota0_sb[:], wota0[:])
        nc.sync.dma_start(wota1_sb[:], wota1[:])
        for g in range(1, QG):
            if g > 1:
                nc.sync.dma_start(
                    xtq_sb[:, :, g * QGS:(g + 1) * QGS],
                    xtq[:, :, g * QGS:(g + 1) * QGS])
            nc.sync.dma_start(
                xt_sb[:, :, 2 * g * QGS:(2 * g + 1) * QGS],
                xt[:, :, 2 * g * QGS:(2 * g + 1) * QGS])
            nc.sync.dma_start(
                xt_sb[:, :, (2 * g + 1) * QGS:(2 * g + 2) * QGS],
                xt[:, :, (2 * g + 1) * QGS:(2 * g + 2) * QGS])

        # ones columns of V (denominator) and of stk1 (bias row of out-proj)
        v1_v = v1_sb[:].rearrange("p k (h x) -> p k h x", x=65)
        nc.vector.memset(v1_v[:, :, :, 64:65], 1.0)
        nc.vector.memset(stk1_sb[64:65, :], 1.0)
        warm_sb = const.tile([1, QGS], bf16, tag="warm")
        nc.vector.memset(warm_sb[:], 0.0)

        # Slot order [head0, head2, head1]: the two base-partition-0 heads
        # share the first PSUM bank of the scores tile; the base-64 head
        # (kt01/qt01 rows 64:128) gets the second bank alone.  Matmuls that
        # share a PSUM bank must use the same operand base partition (HW
        # exec-unit constraint); V and Wo are permuted on the host to match.
        kq = [(kt01_sb, 0), (kt2_sb, 0), (kt01_sb, 64)]
        qq = [(qt01_sb, 0), (qt2_sb, 0), (qt01_sb, 64)]

        with tc.tile_pool(name="st_ps", bufs=2, space="PSUM") as stps, \
             tc.tile_pool(name="pv_ps", bufs=1, space="PSUM") as pvps, \
             tc.tile_pool(name="aux_ps", bufs=2, space="PSUM") as auxps, \
             tc.tile_pool(name="pt", bufs=8) as ptpool, \
             tc.tile_pool(name="nrm", bufs=4) as nrmpool, \
             tc.tile_pool(name="attn", bufs=4) as attnpool, \
             tc.tile_pool(name="oev", bufs=4) as oevpool:

            # per-group PV accumulators (persist across the m-loop)
            pvA = pvps.tile([P, QGS], f32, tag="pvA", name="pvA")  # chunks 0,1
            pvB = pvps.tile([P, QGS], f32, tag="pvB", name="pvB")  # chunks 2,3

            # ---------- projection pieces (emitted interleaved) ----------
            def qk_chain(w_sb, x_sb, x0, dst, bias_col, rows, wlo, name,
                         width=QGS, hoist=0):
                """Returns 2 sub-lump closures (3+3 matmuls) for one
                projection chain, so pacing never inserts a >700ns PE lump."""
                state = {}

                def p1():
                    with tc.high_priority(hoist) if hoist else _nullcm():
                        ps = auxps.tile([P, width], f32, tag="aux", name=name)
                        state["ps"] = ps
                        for c in range(3):
                            nc.tensor.matmul(
                                ps[0:rows, :], w_sb[:, c, wlo:wlo + rows],
                                x_sb[:, c, x0:x0 + width],
                                start=(c == 0), stop=False)

                def p2():
                    with tc.high_priority(hoist) if hoist else _nullcm():
                        ps = state["ps"]
                        for c in range(3, NC):
                            nc.tensor.matmul(
                                ps[0:rows, :], w_sb[:, c, wlo:wlo + rows],
                                x_sb[:, c, x0:x0 + width],
                                start=False, stop=(c == NC - 1))
                    # the evict gates downstream scores: keep it ahead of
                    # other DVE work
                    with tc.high_priority(150):
                        nc.vector.tensor_scalar(dst, ps[0:rows, :], bias_col,
                                                None, add)
                return [p1, p2]

            def q_subs(g, part):
                if part == 0:
                    return qk_chain(wq_sb, xtq_sb, g * QGS,
                                    qt01_sb[:, g * QGS:(g + 1) * QGS],
                                    bias_sb[:, 0:1], 128, 0, f"q{g}_0")
                return qk_chain(wq_sb, xtq_sb, g * QGS,
                                qt2_sb[:, g * QGS:(g + 1) * QGS],
                                bias_sb[0:64, 1:2], 64, 128, f"q{g}_1")

            def k_subs(kg, part):
                if part == 0:
                    return qk_chain(wk_sb, xt_sb, kg * QGS,
                                    kt01_sb[:, kg * QGS:(kg + 1) * QGS],
                                    bias_sb[:, 2:3], 128, 0, f"k{kg}_0")
                return qk_chain(wk_sb, xt_sb, kg * QGS,
                                kt2_sb[:, kg * QGS:(kg + 1) * QGS],
                                bias_sb[0:64, 3:4], 64, 128, f"k{kg}_1")

            def v_piece(kb):
                ps = auxps.tile([P, GD], f32, tag="aux", name=f"v{kb}")
                for c in range(NC):
                    nc.tensor.matmul(
                        ps[:], xt_sb[:, c, kb * P:(kb + 1) * P], wv_sb[:, c, :],
                        start=(c == 0), stop=(c == NC - 1))
                dst = v1_v[:, kb, :, 0:64]
                src = ps[:].rearrange("p (h x) -> p h x", x=64)
                nc.vector.tensor_copy(dst, src)

            # ---------- per-chunk finish: normalize, transpose, out-proj ----------
            def finish_head(qg, j):
                """Reads the chunk's PSUM accumulator: normalize, transpose,
                stack.  MUST be emitted before the next group re-opens the
                bank (WAR)."""
                half, jj = j // 2, j % 2
                pvt = pvA if half == 0 else pvB
                joff = 256 * jj
                view = pvt[:, joff:joff + GH * 65].rearrange(
                    "p (h x) -> p h x", x=65)
                rc = nrmpool.tile([P, GH, 1], f32, tag="rc")
                nc.vector.reciprocal(rc[:], view[:, :, 64:65])
                att = attnpool.tile([P, GD], bf16, tag="attn")
                for h in range(GH):
                    nc.vector.tensor_scalar(
                        att[:, h * HD:(h + 1) * HD], view[:, h, 0:64],
                        rc[:, h, :], None, mult)
                tp = auxps.tile([P, 2 * P], bf16, tag="aux", name=f"tp{qg}_{j}")
                nc.tensor.transpose(tp[:, 0:P], att[:, 0:P], ident_sb[:])
                nc.tensor.transpose(tp[0:64, P:2 * P], att[:, P:GD], ident_sb[:])
                # last group: Act is idle, shorten the tail-critical chain
                if qg == QG - 1:
                    nc.scalar.activation(stk0_sb[:, j * P:(j + 1) * P],
                                         tp[:, 0:P], Copy)
                else:
                    nc.vector.tensor_copy(stk0_sb[:, j * P:(j + 1) * P],
                                          tp[:, 0:P])
                nc.vector.tensor_copy(stk1_sb[0:64, j * P:(j + 1) * P],
                                      tp[0:64, P:2 * P])

            def finish_tail(qg, j, dh):
                """Out-projection half dh for query tile j (reads stk only)."""
                oe_key = (qg, j)
                if oe_key not in oe_tiles:
                    oe_tiles[oe_key] = oevpool.tile([P, D], f32, tag="oe",
                                                    name=f"oe{qg}_{j}")
                oe = oe_tiles[oe_key]
                jq = 4 * qg + j
                op = auxps.tile([P, QGS], f32, tag="aux", name=f"op{qg}_{j}_{dh}")
                nc.tensor.matmul(
                    op[:, 0:384], stk0_sb[:, j * P:(j + 1) * P],
                    wota0_sb[:, dh * 384:(dh + 1) * 384],
                    start=True, stop=False)
                nc.tensor.matmul(
                    op[:, 0:384], stk1_sb[:, j * P:(j + 1) * P],
                    wota1_sb[:, dh * 384:(dh + 1) * 384],
                    start=False, stop=True)
                if qg == QG - 1 and dh == 1:
                    nc.scalar.activation(oe[:, dh * 384:(dh + 1) * 384],
                                         op[:, 0:384], Copy)
                else:
                    nc.vector.tensor_copy(oe[:, dh * 384:(dh + 1) * 384],
                                          op[:, 0:384])
                nc.sync.dma_start(
                    out[jq * P:(jq + 1) * P, dh * 384:(dh + 1) * 384],
                    oe[:, dh * 384:(dh + 1) * 384])

            oe_tiles = {}

            # ---------- main loop ----------
            # PE warm-up: the p-state ramp needs ~3us of continuous work to
            # reach full clock; burn it on dummy matmuls while DMAs stream so
            # the first real projection chains run at full speed.
            for i in range(3):
                wps = auxps.tile([1, QGS], f32, tag="aux", name=f"warm{i}")
                nc.tensor.matmul(wps[:], warm_sb[0:1, 0:1], warm_sb[:],
                                 start=True, stop=True)

            # Upfront projections: only the slices the FIRST scores tile needs
            # (Q cols 0:256, K cols 0:128), so the first exp issues ~6us in.
            # The rest streams in via unit pre/post slots below.
            for fn in (qk_chain(wq_sb, xtq_sb, 0, qt01_sb[:, 0:HS],
                                bias_sb[:, 0:1], 128, 0, "qa0", HS)
                       + qk_chain(wq_sb, xtq_sb, 0, qt2_sb[:, 0:HS],
                                  bias_sb[0:64, 1:2], 64, 128, "qa1", HS)
                       + qk_chain(wk_sb, xt_sb, 0, kt01_sb[:, 0:P],
                                  bias_sb[:, 2:3], 128, 0, "ka0", P)
                       + qk_chain(wk_sb, xt_sb, 0, kt2_sb[:, 0:P],
                                  bias_sb[0:64, 3:4], 64, 128, "ka1", P)):
                fn()

            # Build the full tile sequence with per-tile emission closures, then
            # emit with a one-stage software-pipeline skew: scores+exp of tile
            # i+1 go into the engine queues BEFORE mask+PV of tile i, so the
            # in-order PE queue never parks on a PV that waits for its exp.
            units = []   # list of dicts: front(), back(), post list

            def make_front(qg, m, half, lo, holder):
                def front():
                    # scores+exp are the pacing stream: keep them ahead of
                    # filler in the scheduler's priority heap so a ready
                    # scores matmul is never queued behind projection lumps
                    with tc.high_priority(800):
                        qbase = qg * QGS + half * HS
                        st = stps.tile([P, GH, HS], f32, tag="st",
                                       name=f"st{qg}_{m}_{half}")
                        for h in range(GH):
                            kt_t, kp = kq[h]
                            qt_t, qp = qq[h]
                            nc.tensor.matmul(
                                st[:, h, lo:], kt_t[kp:kp + 64, m * P:(m + 1) * P],
                                qt_t[qp:qp + 64, qbase + lo:qbase + HS],
                                start=True, stop=True)
                        pt = ptpool.tile([P, GH, HS], bf16, tag="pt")
                        nc.scalar.activation(pt[:, :, lo:], st[:, :, lo:], Exp)
                    holder.append(pt)
                return front

            def make_back(qg, m, half, jlo, jhi, q0b, holder):
                def back():
                    pt = holder[0]
                    w = m - 8 * qg
                    if w >= 0:
                        # multiplicative 0/1 causal mask on boundary blocks;
                        # hoisted above other DVE work (it gates this tile's PV)
                        glo = max(P * q0b, half * HS)
                        ghi = min(P * (w // 2 + 1), half * HS + HS)
                        if glo < ghi:
                            a, b = glo - half * HS, ghi - half * HS
                            with tc.high_priority(150):
                                for h in range(GH):
                                    nc.vector.tensor_tensor(
                                        pt[:, h, a:b], pt[:, h, a:b],
                                        mask_sb[:, w, glo:ghi], mult)
                    pvt = pvA if half == 0 else pvB
                    for j in range(jlo, jhi + 1):
                        jj = j - 2 * half
                        for h in range(GH):
                            # one accumulation group per PSUM bank: start only
                            # on the bank's very first write (it marks the
                            # whole 2KB zero-region), stop only on the bank's
                            # very last write.
                            first = (m == 0 and j == 2 * half and h == 0)
                            last = (j == 2 * half + 1 and h == GH - 1
                                    and m == 8 * qg + 2 * j + 1)
                            nc.tensor.matmul(
                                pvt[:, 256 * jj + 65 * h:256 * jj + 65 * h + 65],
                                pt[:, h, jj * P:(jj + 1) * P],
                                v1_sb[:, m, 65 * h:65 * h + 65],
                                start=first, stop=last)
                return back

            # pass 1: enumerate tiles
            idx_of = {}
            for qg in range(QG):
                kcnt = 8 * (qg + 1)
                for m in range(kcnt):
                    q0b = _q0b(qg, m)
                    for half in range(2):
                        jlo = max(2 * half, q0b)
                        jhi = 2 * half + 1
                        if jlo > jhi:
                            continue
                        lo = P * (jlo - 2 * half)
                        holder = []
                        units.append({
                            "front": make_front(qg, m, half, lo, holder),
                            "back": make_back(qg, m, half, jlo, jhi, q0b, holder),
                            "pre": [],
                            "post": [],
                        })
                    idx_of[(qg, m)] = len(units) - 1

            # pass 2: attach finishes and spread projection pieces
            def attach(idx, fn):
                units[min(idx, len(units) - 1)]["post"].append(fn)

            for qg in range(QG):
                # pvA closes mid-group; spread its finish work over the next
                # units so no single PE lump starves the Act exp stream.
                closeA = 8 * qg + 3 + (3 if qg == 0 else 0)
                ia = idx_of[(qg, closeA)]
                attach(ia, lambda qg=qg: finish_head(qg, 0))
                attach(ia + 1, lambda qg=qg: finish_head(qg, 1))
                for k, (j, dh) in enumerate([(0, 0), (0, 1), (1, 0), (1, 1)]):
                    attach(ia + 2 + k,
                           lambda qg=qg, j=j, dh=dh: finish_tail(qg, j, dh))
                # pvB closes at group end.  Its heads must still be emitted
                # before the NEXT group's first half-1 PV re-opens the bank,
                # i.e. no later than the post of unit (g+1, m=0, half0).
                ib = idx_of[(qg, 8 * qg + 7)]
                attach(ib + 1, lambda qg=qg: finish_head(qg, 2))
                attach(ib + 1, lambda qg=qg: finish_head(qg, 3))
                for k, (j, dh) in enumerate([(2, 0), (2, 1), (3, 0), (3, 1)]):
                    attach(ib + 2 + k,
                           lambda qg=qg, j=j, dh=dh: finish_tail(qg, j, dh))

            # group-boundary blackout: no filler right where the next group's
            # first scores must reach the PE queue head
            black = set()
            for g in range(QG):
                ib = idx_of[(g, 8 * g + 7)]
                black |= {ib - 1, ib, ib + 1}

            def spread(pieces, i0, i1):
                """Distribute sub-lumps uniformly over unit posts [i0, i1],
                respecting per-piece deadlines and boundary blackouts."""
                n = len(pieces)
                for k, (fn, dl) in enumerate(pieces):
                    u = min(i0 + (k * (i1 - i0 + 1)) // n, dl)
                    while u in black and u > i0:
                        u -= 1
                    units[u]["post"].append(fn)

            # startup remainder chains in unit pre-slots (pre of unit i runs
            # right after front(i), i.e. before front(i+1) and back(i)):
            # Q cols 256:512 after tile 0's scores; K cols 128:256 before m=1;
            # K cols 256:512 before m=2; V(kb) before back of m=kb.
            units[0]["pre"] += (
                qk_chain(wq_sb, xtq_sb, HS, qt01_sb[:, HS:QGS],
                         bias_sb[:, 0:1], 128, 0, "qb0", HS)
                + qk_chain(wq_sb, xtq_sb, HS, qt2_sb[:, HS:QGS],
                           bias_sb[0:64, 1:2], 64, 128, "qb1", HS))
            units[1]["pre"] += (
                qk_chain(wk_sb, xt_sb, P, kt01_sb[:, P:2 * P],
                         bias_sb[:, 2:3], 128, 0, "kb0", P)
                + qk_chain(wk_sb, xt_sb, P, kt2_sb[:, P:2 * P],
                           bias_sb[0:64, 3:4], 64, 128, "kb1", P)
                + [lambda: v_piece(0)])
            units[3]["pre"] += (
                qk_chain(wk_sb, xt_sb, 2 * P, kt01_sb[:, 2 * P:QGS],
                         bias_sb[:, 2:3], 128, 0, "kc0", 2 * P)
                + qk_chain(wk_sb, xt_sb, 2 * P, kt2_sb[:, 2 * P:QGS],
                           bias_sb[0:64, 3:4], 64, 128, "kc1", 2 * P)
                + [lambda: v_piece(1)])
            units[5]["pre"].append(lambda: v_piece(2))
            units[7]["pre"].append(lambda: v_piece(3))

            # group 0's remaining K/V: held until their xt/wv DMAs land
            own0 = [(s, idx_of[(0, 3)]) for s in k_subs(1, 0) + k_subs(1, 1)]
            own0 += [(lambda kb=kb: v_piece(kb), idx_of[(0, kb - 1)])
                     for kb in range(4, 8)]
            spread(own0, idx_of[(0, 1)] + 1, idx_of[(0, 5)])

            # groups 1..3: K/V spread from late in the previous group across
            # this group's pre-boundary iterations (window start respects the
            # xt DMA arrival order); per-piece deadline = first consumer.
            kv_start = {1: (1, 4), 2: (2, 0), 3: (3, 0)}
            for g in range(1, QG):
                pieces = [(s, idx_of[(g, 8 * g - 1)])
                          for s in k_subs(2 * g, 0) + k_subs(2 * g, 1)]
                pieces += [(s, idx_of[(g, 8 * g + 3)])
                           for s in k_subs(2 * g + 1, 0) + k_subs(2 * g + 1, 1)]
                pieces += [(lambda kb=kb: v_piece(kb), idx_of[(g, kb - 1)])
                           for kb in range(8 * g, 8 * g + 8)]
                sg, sm = kv_start[g]
                spread(pieces, idx_of[(sg, sm)], idx_of[(g, 8 * g + 2)])
                # Q(g) as soon as its xtq DMA has landed — far away from the
                # group boundary so the first scores of group g issue promptly
                q0g, q0m, q1m = {1: (0, 5, 6), 2: (1, 2, 5), 3: (2, 0, 3)}[g]
                qp = [(s, idx_of[(q0g, q1m + 1)])
                      for s in q_subs(g, 0) + q_subs(g, 1)]
                spread(qp, idx_of[(q0g, q0m)], idx_of[(q0g, q1m)])

            # skewed emission
            prev = None
            for u in units:
                u["front"]()
                for fn in u["pre"]:
                    fn()
                if prev is not None:
                    prev["back"]()
                    for fn in prev["post"]:
                        fn()
                prev = u
            prev["back"]()
            for fn in prev["post"]:
                fn()

    nc.compile()
    return nc


def _host_prep(inputs, Wq, bq, Wk, bk, Wv, bv, Wo, bo):
    import ml_dtypes

    bf16 = ml_dtypes.bfloat16
    X = np.asarray(inputs, np.float32).reshape(S, D)
    XT = np.ascontiguousarray(X.T)                      # [768, 4096]
    # partition-major: [128, 6, S]
    XTp = np.ascontiguousarray(
        XT.reshape(NC, P, S).transpose(1, 0, 2)).astype(bf16)
    XTb = XT.reshape(D, NKB // 2, 2, P)
    XTqp = []
    for s_ in range(2):
        XTq = XTb[:, :, s_, :].reshape(D, SL)
        XTqp.append(np.ascontiguousarray(
            XTq.reshape(NC, P, SL).transpose(1, 0, 2)).astype(bf16))

    # per-core multiplicative causal masks [128, 8*512], 1=keep 0=drop
    # (S_T layout: k on partitions, q on free dim)
    tri = (np.arange(P)[None, :] >= np.arange(P)[:, None]).astype(np.float32)
    mk = []
    for s_ in range(2):
        m = np.ones((P, 8, QGS), np.float32)
        for w in range(8):
            npref = max(0, -(-(w - s_) // 2))  # ceil((w - s)/2) clamped at 0
            m[:, w, :P * npref] = 0.0
            if w >= s_ and (w - s_) % 2 == 0:
                dblk = (w - s_) // 2
                m[:, w, dblk * P:(dblk + 1) * P] = tri
        mk.append(np.ascontiguousarray(m.reshape(P, 8 * QGS)).astype(bf16))

    idm = np.eye(P, dtype=np.float32).astype(bf16)

    def pmajor(WT):  # [768, 192] -> [128, 6, 192]
        return np.ascontiguousarray(
            WT.reshape(NC, P, GD).transpose(1, 0, 2)).astype(bf16)

    in_maps = []
    for g in range(NG):
        hs = slice(GD * g, GD * (g + 1))
        WqT = pmajor(np.ascontiguousarray(Wq[hs, :].T) / 8.0)
        WkT = pmajor(np.ascontiguousarray(Wk[hs, :].T))
        # slot order [head0, head2, head1] (see kq/qq comment in the program)
        sperm = np.r_[0:64, 128:192, 64:128]
        WvT = pmajor(np.ascontiguousarray(Wv[hs, :].T[:, sperm]))
        WoT = np.ascontiguousarray(Wo[:, hs].T[sperm, :]).astype(np.float32)
        bo_g = bv[hs][sperm].astype(np.float32) @ WoT
        if g == 0:
            bo_g = bo_g + bo.astype(np.float32)
        wota = np.concatenate([WoT, bo_g[None, :]], axis=0)  # [193, 768]
        wota0 = np.ascontiguousarray(wota[0:P]).astype(bf16)
        wota1 = np.ascontiguousarray(wota[P:]).astype(bf16)
        bias_t = np.zeros((P, 4), np.float32)
        bias_t[:, 0] = bq[hs][0:128] / 8.0
        bias_t[0:64, 1] = bq[hs][128:192] / 8.0
        bias_t[:, 2] = bk[hs][0:128]
        bias_t[0:64, 3] = bk[hs][128:192]
        for s_ in range(2):
            in_maps.append({
                "xt": XTp, "xtq": XTqp[s_],
                "wq": WqT, "wk": WkT, "wv": WvT,
                "wota0": wota0, "wota1": wota1,
                "bias": bias_t, "masks": mk[s_], "ident": idm,
            })
    return in_maps


def _gather(results):
    out = np.zeros((S, D), np.float32)
    ov = out.reshape(NQB, 2, P, D)
    for s_ in range(2):
        acc = np.zeros((SL, D), np.float32)
        for g in range(NG):
            acc += np.asarray(results[2 * g + s_]["out"], np.float32)
        ov[:, s_, :, :] = acc.reshape(NQB, P, D)
    return out.reshape(1, S, D)


def kernel(inputs, Wq, bq, Wk, bk, Wv, bv, Wo, bo):
    from concourse.bass_utils import run_bass_kernel_spmd

    if "nc" not in _CACHE:
        _CACHE["nc"] = _build_program()
    nc = _CACHE["nc"]
    in_maps = _host_prep(
        np.asarray(inputs), np.asarray(Wq), np.asarray(bq), np.asarray(Wk),
        np.asarray(bk), np.asarray(Wv), np.asarray(bv), np.asarray(Wo),
        np.asarray(bo))
    res = run_bass_kernel_spmd(nc, in_maps, list(range(8))).results
    return _gather(res)
